# revision 27
# baseline (speedup 1.0000x reference)
"""Causal self-attention kernel for 8 Trainium2 NeuronCores.

Problem: B=2, T=2048, D=2048, H=16, Dh=128, fp32 in/out.
  qkv = x @ Wqkv + bqkv ; per-head causal attention ; out = att @ Wout + bout

Sharding (tensor parallel over heads + AllToAll before out_proj):
  Core c owns heads {2c, 2c+1}. Each core computes Q^T/K^T (head-dim on
  partitions) and V (token-dim on partitions) for all 4096 tokens via the
  QKV projection with its 768-column shard of Wqkv, runs causal attention
  locally (scores computed transposed: S^T[k,q], softmax reduction over k
  via an all-ones matmul which also broadcasts the denominator), and
  produces att^T per batch. Four AllToAlls (one per half-batch of tokens)
  redistribute head-sharded -> token-sharded; core c projects its 128-token
  slices with the full Wout (resident in SBUF).

Schedule: flash-style interleave. Attention group (hl, qc) is emitted as
soon as proj chunks covering tokens <= (qc+1)*512 land, so AllToAlls fire
mid-phase; batch-0's out-projection runs inside batch-1's proj/attention
phase, leaving only batch-1's out-projection in the tail.

All matmul operands are bf16 (fp32 PSUM accumulation); softmax denominators
use reciprocal_approx_fast (fp32, ~18-bit).
"""

import numpy as np
import ml_dtypes

import concourse.bass as bass
import concourse.mybir as mybir
import concourse.tile as tile
from concourse import bacc
from concourse.bass_utils import run_bass_kernel_spmd

B, T, D, H, Dh = 2, 2048, 2048, 16, 128
NT = B * T                  # 4096 tokens total
W = 8                       # cores
HL = H // W                 # 2 heads per core
CQKV = 3 * HL * Dh          # 768 qkv columns per core
KO = D // 128               # 16 contraction subtiles
TC = 256                    # proj token chunk
NTC_B = T // TC             # 8 chunks per batch
QC = 512                    # attention q-chunk
NQC = T // QC               # 4 q-chunks per batch
HT = T // 2                 # half-batch token span (one AllToAll each)
TOKH = HT // W              # 128 tokens per core per half-batch exchange
SCALE = 1.0 / float(np.sqrt(Dh))

F32 = mybir.dt.float32
BF16 = mybir.dt.bfloat16
FP8 = mybir.dt.float8e4
DR = mybir.MatmulPerfMode.DoubleRow
EXPB = -2.0                 # exp bias shift: keeps exp(s+EXPB) < fp8e4 max (240)
MULT = mybir.AluOpType.mult
ADD = mybir.AluOpType.add


def _build():
    nc = bacc.Bacc("TRN2", target_bir_lowering=False, debug=False,
                   enable_asserts=True, num_devices=W)
    xT = nc.dram_tensor("xT", [D, NT], BF16, kind="ExternalInput").ap()
    wqkv = nc.dram_tensor("wqkv", [D, CQKV], BF16, kind="ExternalInput").ap()
    bqkv = nc.dram_tensor("bqkv", [2 * HL * 128], F32, kind="ExternalInput").ap()
    wout = nc.dram_tensor("wout", [D, D], BF16, kind="ExternalInput").ap()
    maskneg = nc.dram_tensor("maskneg", [128, 128], BF16, kind="ExternalInput").ap()
    bvbc = nc.dram_tensor("bvbc", [128, 2 * HL * Dh], F32, kind="ExternalInput").ap()
    boutbc = nc.dram_tensor("boutbc", [128, D], F32, kind="ExternalInput").ap()
    # rows [(b*2+half)*TOKH ...): tokens [half*HT + c*TOKH ...) of batch b
    out = nc.dram_tensor("out", [B * 2 * TOKH, D], F32, kind="ExternalOutput").ap()

    xT_v = xT.rearrange("(ko p) t -> p ko t", p=128)
    wqkv_v = wqkv.rearrange("(ko p) c -> p ko c", p=128)
    wout_v = wout.rearrange("(ko p) c -> p ko c", p=128)

    with tile.TileContext(nc) as tc:
        with tc.tile_pool(name="persist", bufs=1) as persist, \
             tc.tile_pool(name="dram", bufs=1, space="DRAM") as dram_pool:
            mask_sb = persist.tile([128, 128], BF16, name="mask")   # 0 / -1e9
            ones8_sb = persist.tile([128, 2, 128], BF16, name="ones8")
            bqk_sb = persist.tile([128, 2 * HL], F32, name="bqk")
            expb_sb = persist.tile([128, 1], F32, name="expb")
            bv_sb = persist.tile([128, 2 * HL * Dh], F32, name="bv")  # (hl tb d)
            bout_sb = persist.tile([128, D], F32, name="bout")
            wqkv_sb = [persist.tile([128, CQKV], BF16, name=f"wqkv{ko}")
                       for ko in range(KO)]
            wout_sb = [persist.tile([128, D], BF16, name=f"wout{ko}")
                       for ko in range(KO)]

            # small constants + qkv weights first (needed immediately)
            nc.gpsimd.memset(expb_sb[:], EXPB)
            nc.gpsimd.memset(ones8_sb[:], 1.0)
            nc.sync.dma_start(mask_sb[:], maskneg)
            nc.sync.dma_start(bqk_sb[:], bqkv.rearrange("(cc p) -> p cc", p=128))
            nc.sync.dma_start(bv_sb[:], bvbc)

            a2a_in = [[dram_pool.tile([W, HL * 128, TOKH], BF16, name=f"a2a_in{b}{h}")
                       for h in range(2)] for b in range(B)]
            a2a_out = [[dram_pool.tile([W, HL * 128, TOKH], BF16, name=f"a2a_out{b}{h}")
                        for h in range(2)] for b in range(B)]

            with tc.tile_pool(name="x_pool", bufs=2) as x_pool, \
                 tc.tile_pool(name="ex_pool", bufs=3) as ex_pool, \
                 tc.tile_pool(name="rden_pool", bufs=2) as rden_pool, \
                 tc.tile_pool(name="attc_pool", bufs=3) as attc_pool, \
                 tc.tile_pool(name="attall_pool", bufs=4) as attall_pool, \
                 tc.tile_pool(name="o_pool", bufs=3) as o_pool, \
                 tc.tile_pool(name="proj_psum", bufs=2, space="PSUM") as proj_psum, \
                 tc.tile_pool(name="s_psum", bufs=2, space="PSUM") as s_psum, \
                 tc.tile_pool(name="av_psum", bufs=2, space="PSUM") as av_psum, \
                 tc.tile_pool(name="dout_psum", bufs=2, space="PSUM") as dout_psum:

                def prefetch_x(b, ci):
                    t0 = b * T + ci * TC
                    x_sb = x_pool.tile([128, KO, TC], BF16, name="x_sb")
                    nc.sync.dma_start(x_sb[:], xT_v[:, :, t0:t0 + TC])
                    return x_sb

                def emit_proj_chunk(qkv, b, ci, x_pre=None):
                    """Project one 512-token chunk of batch b into qT/kT/v."""
                    qTb, kTb, vb = qkv
                    x_sb = x_pre if x_pre is not None else prefetch_x(b, ci)
                    for cc in range(2 * HL):        # Q_h0, Q_h1, K_h0, K_h1
                        ps = proj_psum.tile([128, TC], F32, name="proj_ps")
                        for ko in range(KO):
                            nc.tensor.matmul(
                                ps[:],
                                wqkv_sb[ko][:, cc * 128:(cc + 1) * 128],
                                x_sb[:, ko, :],
                                start=(ko == 0), stop=(ko == KO - 1))
                        dest = qTb if cc < HL else kTb
                        hl = cc if cc < HL else cc - HL
                        nc.vector.tensor_scalar_add(
                            dest[:, hl, ci * TC:(ci + 1) * TC], ps[:],
                            bqk_sb[:, cc:cc + 1])
                    for tb in range(TC // 128):
                        ps = proj_psum.tile([128, TC], F32, name="proj_ps")
                        for ko in range(KO):
                            nc.tensor.matmul(
                                ps[:],
                                x_sb[:, ko, tb * 128:(tb + 1) * 128],
                                wqkv_sb[ko][:, 2 * HL * 128:],
                                start=(ko == 0), stop=(ko == KO - 1))
                        vidx = ci * (TC // 128) + tb
                        nc.vector.tensor_tensor(
                            vb[:, :, vidx, :],
                            ps[:].rearrange("p (hl d) -> p hl d", hl=HL),
                            bv_sb[:].rearrange("p (hl tb d) -> p hl tb d",
                                               hl=HL, tb=2)[:, :, tb, :],
                            ADD)

                def emit_attn_group(qkv, b, hl, qc):
                    qTb, kTb, vb = qkv
                    """One (head, q-chunk) group: S^T -> exp -> P^T V, denom via
                    ones-matmul; normalized att^T chunk DMAed to a2a_in.

                    Off-diagonal k-blocks are processed in pairs as fp8
                    DoubleRow matmuls (2x PE rate); diagonal blocks get an
                    additive -1e9 causal mask on the fp32 scores pre-exp."""
                    q0 = qc * QC
                    nkb = (qc + 1) * (QC // 128)
                    ndiag = QC // 128
                    npair = (nkb - ndiag) // 2
                    ps_av = av_psum.tile([128, QC], F32, name="ps_av")
                    ps_d = dout_psum.tile([128, QC], F32, name="ps_do")
                    units = [("pair", 2 * i) for i in range(npair)] \
                        + [("diag", 2 * npair + j) for j in range(ndiag)]
                    exs = {}

                    def emit_S_unit(u):
                        kind, kb = u
                        if kind == "pair":
                            ex2 = ex_pool.tile([128, 2, QC], BF16, name="ex2")
                            for t in range(2):
                                ps_s = s_psum.tile([128, QC], F32, name="ps_s")
                                nc.tensor.matmul(
                                    ps_s[:],
                                    kTb[:, hl, (kb + t) * 128:(kb + t + 1) * 128],
                                    qTb[:, hl, q0:q0 + QC],
                                    start=True, stop=True)
                                nc.scalar.activation(
                                    ex2[:, t, :], ps_s[:],
                                    mybir.ActivationFunctionType.Exp,
                                    scale=SCALE, bias=expb_sb[:])
                            exs[u] = ex2
                        else:
                            vs = (kb - qc * ndiag) * 128
                            ps_s = s_psum.tile([128, QC], F32, name="ps_s")
                            nc.tensor.matmul(
                                ps_s[:, vs:], kTb[:, hl, kb * 128:(kb + 1) * 128],
                                qTb[:, hl, q0 + vs:q0 + QC], start=True, stop=True)
                            nc.vector.tensor_tensor(
                                ps_s[:, vs:vs + 128], ps_s[:, vs:vs + 128],
                                mask_sb[:], ADD)
                            ex = ex_pool.tile([128, QC], BF16, name="ex")
                            nc.scalar.activation(
                                ex[:, vs:], ps_s[:, vs:],
                                mybir.ActivationFunctionType.Exp,
                                scale=SCALE, bias=expb_sb[:])
                            exs[u] = (ex, vs)

                    def emit_PV_unit(u, first, last):
                        kind, kb = u
                        if kind == "pair":
                            ex2 = exs[u]
                            nc.tensor.matmul(
                                ps_av[:], vb[:, hl, kb, :], ex2[:, 0, :],
                                start=first, stop=False)
                            nc.tensor.matmul(
                                ps_av[:], vb[:, hl, kb + 1, :], ex2[:, 1, :],
                                start=False, stop=last)
                            nc.tensor.matmul(
                                ps_d[:], ones8_sb[:, 0, :], ex2[:, 0, :],
                                start=first, stop=False)
                            nc.tensor.matmul(
                                ps_d[:], ones8_sb[:, 1, :], ex2[:, 1, :],
                                start=False, stop=last)
                        else:
                            ex, vs = exs[u]
                            nc.tensor.matmul(
                                ps_av[:, vs:], vb[:, hl, kb, :], ex[:, vs:],
                                start=first, stop=last)
                            nc.tensor.matmul(
                                ps_d[:, vs:], ones8_sb[:, 0, :], ex[:, vs:],
                                start=first, stop=last)

                    emit_S_unit(units[0])
                    for j in range(1, len(units)):
                        emit_S_unit(units[j])
                        emit_PV_unit(units[j - 1], j == 1, False)
                    emit_PV_unit(units[-1], len(units) == 1, True)

                    rden = rden_pool.tile([128, QC], F32, name="rden")
                    nc.vector.reciprocal_approx_fast(rden[:], ps_d[:])
                    attc = attc_pool.tile([128, QC], BF16, name="attc")
                    nc.vector.tensor_tensor(attc[:], ps_av[:], rden[:], MULT)
                    h = qc // 2
                    view = a2a_in[b][h].rearrange(
                        "(hh rr) (hl p) t -> p hl hh rr t",
                        hh=2, rr=W // 2, hl=HL, p=128)
                    nc.gpsimd.dma_start(
                        view[:, hl, qc % 2],
                        attc[:].rearrange("p (rr t) -> p rr t", rr=W // 2))

                def emit_a2a(b, h):
                    nc.gpsimd.collective_compute(
                        "AllToAll", mybir.AluOpType.bypass,
                        replica_groups=[list(range(W))],
                        ins=[a2a_in[b][h][:].opt()], outs=[a2a_out[b][h][:].opt()])

                def emit_attall(b, h, slot):
                    ga = attall_pool.tile([128, KO, TOKH], BF16, name="attall")
                    nc.sync.dma_start(
                        ga[:],
                        a2a_out[b][h].rearrange("r (hl p) t -> p (r hl) t",
                                                hl=HL, p=128))
                    slot[(b, h)] = ga

                def emit_outproj(b, h, slot):
                    ga = slot[(b, h)]
                    for colc in range(D // 512):
                        ps_o = dout_psum.tile([128, 512], F32, name="ps_do")
                        for ko in range(KO):
                            nc.tensor.matmul(
                                ps_o[:], ga[:, ko, :],
                                wout_sb[ko][:, colc * 512:(colc + 1) * 512],
                                start=(ko == 0), stop=(ko == KO - 1))
                        o_sb = o_pool.tile([128, 512], F32, name="o_sb")
                        nc.vector.tensor_tensor(
                            o_sb[:], ps_o[:],
                            bout_sb[:, colc * 512:(colc + 1) * 512], ADD)
                        nc.sync.dma_start(
                            out[(b * 2 + h) * TOKH:(b * 2 + h + 1) * TOKH,
                                colc * 512:(colc + 1) * 512],
                            o_sb[:])

                ga = {}
                # x chunk 0 DMA first (1MB, gates the first matmul), then the
                # wqkv tiles; wout (8MB) is held back until proj chunk 2's
                # output exists so it can't steal startup HBM bandwidth
                x0_sb = x_pool.tile([128, KO, TC], BF16, name="x_sb")
                nc.sync.dma_start(x0_sb[:, 0:KO // 2], xT_v[:, 0:KO // 2, 0:TC])
                nc.scalar.dma_start(x0_sb[:, KO // 2:], xT_v[:, KO // 2:, 0:TC])
                x_pre = {(0, 0): x0_sb}
                for ko in range(KO):
                    nc.gpsimd.dma_start(wqkv_sb[ko][:], wqkv_v[:, ko, :])

                def emit_wout_load(qT0, ci):
                    # Tile hoists dependency-free DMAs to t=0, which would let
                    # this 8.5MB steal startup HBM bandwidth from the critical
                    # x/wqkv transfers. Gate each pair of tiles behind a write
                    # that depends on batch-0's qT chunk ci (WAW on the DMA),
                    # spreading the load across the whole batch-0 phase.
                    tq = ci * TC + 1
                    if ci == 0:
                        nc.vector.tensor_scalar_add(
                            bout_sb[:, 0:1], qT0[:, 0, tq:tq + 1], 0.0)
                        nc.scalar.dma_start(bout_sb[:], boutbc)
                    for ko in (2 * ci, 2 * ci + 1):
                        nc.vector.tensor_scalar_add(
                            wout_sb[ko][:, 0:1], qT0[:, 0, tq:tq + 1], 0.0)
                        nc.scalar.dma_start(wout_sb[ko][:], wout_v[:, ko, :])

                def alloc_qkv(pool, b):
                    return (pool.tile([128, HL, T], BF16, name=f"qT{b}"),
                            pool.tile([128, HL, T], BF16, name=f"kT{b}"),
                            pool.tile([128, HL, T // 128, Dh], BF16,
                                      name=f"v{b}"))

                # batch 0: proj + attention interleaved, A2As fire mid-phase.
                # Each batch's qT/kT/v live in their own scoped pool so batch
                # 1 reuses batch 0's SBUF once its attention is done.
                with tc.tile_pool(name="qkv0", bufs=1) as qkv0_pool:
                    qkv0 = alloc_qkv(qkv0_pool, 0)
                    for ci in range(NTC_B):
                        emit_proj_chunk(qkv0, 0, ci, x_pre.get((0, ci)))
                        emit_wout_load(qkv0[0], ci)
                        if ci % 2 == 1:
                            qc = ci // 2
                            emit_attn_group(qkv0, 0, 0, qc)
                            emit_attn_group(qkv0, 0, 1, qc)
                            if qc == 1:
                                emit_a2a(0, 0)
                            if qc == 3:
                                emit_a2a(0, 1)
                # batch 1: same; out-projections all run in the tail
                with tc.tile_pool(name="qkv1", bufs=1) as qkv1_pool:
                    qkv1 = alloc_qkv(qkv1_pool, 1)
                    for ci in range(NTC_B):
                        emit_proj_chunk(qkv1, 1, ci)
                        if ci == 2:
                            emit_attall(0, 0, ga)
                        if ci == 4:
                            emit_attall(0, 1, ga)
                        if ci % 2 == 1:
                            qc = ci // 2
                            emit_attn_group(qkv1, 1, 0, qc)
                            emit_attn_group(qkv1, 1, 1, qc)
                            if qc == 1:
                                emit_a2a(1, 0)
                            if qc == 3:
                                emit_a2a(1, 1)
                    # tail: batch-0's out-projections are independent of
                    # batch-1's A2As, so ~35us of PE work hides the last A2A
                    # before outproj(1,*) needs its data
                    emit_attall(1, 0, ga)
                    emit_attall(1, 1, ga)
                    emit_outproj(0, 0, ga)
                    emit_outproj(0, 1, ga)
                    emit_outproj(1, 0, ga)
                    emit_outproj(1, 1, ga)
    nc.compile()
    return nc


_CACHED_NC = None


def kernel(x, Wqkv, bqkv, Wout, bout):
    global _CACHED_NC
    x = np.asarray(x, dtype=np.float32)
    Wqkv = np.asarray(Wqkv, dtype=np.float32)
    bqkv = np.asarray(bqkv, dtype=np.float32)
    Wout = np.asarray(Wout, dtype=np.float32)
    bout = np.asarray(bout, dtype=np.float32)

    if _CACHED_NC is None:
        _CACHED_NC = _build()
    nc = _CACHED_NC

    bf16 = ml_dtypes.bfloat16
    xT = np.ascontiguousarray(x.reshape(NT, D).T).astype(bf16)   # [D, NT]
    wq4 = Wqkv.reshape(D, 3, H, Dh)                 # col = (which, head, dh)
    bq4 = bqkv.reshape(3, H, Dh)
    kl = np.arange(128)[:, None]
    jl = np.arange(128)[None, :]
    masktri = (jl >= kl).astype(bf16)
    wout_bf = Wout.astype(bf16)
    boutbc = np.tile(bout[None, :], (128, 1)).astype(np.float32)

    in_maps = []
    for c in range(W):
        wshard = np.ascontiguousarray(
            wq4[:, :, HL * c:HL * c + HL, :].reshape(D, CQKV)).astype(bf16)
        bshard_qk = np.ascontiguousarray(
            bq4[0:2, HL * c:HL * c + HL, :].reshape(2 * HL * 128)
        ).astype(np.float32)
        bshard_v = bq4[2, HL * c:HL * c + HL, :]                  # [HL, Dh]
        bvbc = np.ascontiguousarray(np.broadcast_to(
            bshard_v.reshape(1, HL, 1, Dh), (128, HL, 2, Dh)
        ).reshape(128, 2 * HL * Dh)).astype(np.float32)
        in_maps.append({
            "xT": xT, "wqkv": wshard, "bqkv": bshard_qk,
            "wout": wout_bf, "masktri": masktri,
            "ones": np.ones((128, 128), bf16),
            "bvbc": bvbc,
            "boutbc": boutbc,
        })

    res = run_bass_kernel_spmd(nc, in_maps, core_ids=list(range(W)))
    # res[c]["out"] rows [(b*2+h)*TOKH ...) = tokens [h*HT + c*TOKH ...) of batch b
    full = np.empty((B, T, D), np.float32)
    for c in range(W):
        for b in range(B):
            for h in range(2):
                full[b, h * HT + c * TOKH:h * HT + (c + 1) * TOKH] = \
                    res.results[c]["out"][(b * 2 + h) * TOKH:(b * 2 + h + 1) * TOKH]
    return full


# revision 28
# speedup vs baseline: 1.1337x; 1.1337x over previous
"""Causal self-attention kernel for 8 Trainium2 NeuronCores.

Problem: B=2, T=2048, D=2048, H=16, Dh=128, fp32 in/out.
  qkv = x @ Wqkv + bqkv ; per-head causal attention ; out = att @ Wout + bout

Sharding (tensor parallel over heads + AllToAll before out_proj):
  Core c owns heads {2c, 2c+1}. Each core computes Q^T/K^T (head-dim on
  partitions) and V (token-dim on partitions) for all 4096 tokens via the
  QKV projection with its 768-column shard of Wqkv, runs causal attention
  locally (scores computed transposed: S^T[k,q], softmax reduction over k
  via an all-ones matmul which also broadcasts the denominator), and
  produces att^T per batch. Four AllToAlls (one per half-batch of tokens)
  redistribute head-sharded -> token-sharded; core c projects its 128-token
  slices with the full Wout (resident in SBUF).

Schedule: flash-style interleave. Attention group (hl, qc) is emitted as
soon as proj chunks covering tokens <= (qc+1)*512 land, so AllToAlls fire
mid-phase; batch-0's out-projection runs inside batch-1's proj/attention
phase, leaving only batch-1's out-projection in the tail.

All matmul operands are bf16 (fp32 PSUM accumulation); softmax denominators
use reciprocal_approx_fast (fp32, ~18-bit).
"""

import numpy as np
import ml_dtypes

import concourse.bass as bass
import concourse.mybir as mybir
import concourse.tile as tile
from concourse import bacc
from concourse.bass_utils import run_bass_kernel_spmd

B, T, D, H, Dh = 2, 2048, 2048, 16, 128
NT = B * T                  # 4096 tokens total
W = 8                       # cores
HL = H // W                 # 2 heads per core
CQKV = 3 * HL * Dh          # 768 qkv columns per core
KO = D // 128               # 16 contraction subtiles
TC = 256                    # proj token chunk
NTC_B = T // TC             # 8 chunks per batch
QC = 512                    # attention q-chunk
NQC = T // QC               # 4 q-chunks per batch
HT = T // 2                 # half-batch token span (one AllToAll each)
TOKH = HT // W              # 128 tokens per core per half-batch exchange
SCALE = 1.0 / float(np.sqrt(Dh))

F32 = mybir.dt.float32
BF16 = mybir.dt.bfloat16
FP8 = mybir.dt.float8e4
DR = mybir.MatmulPerfMode.DoubleRow
EXPB = -2.0                 # exp bias shift: keeps exp(s+EXPB) < fp8e4 max (240)
MULT = mybir.AluOpType.mult
ADD = mybir.AluOpType.add


def _build():
    nc = bacc.Bacc("TRN2", target_bir_lowering=False, debug=False,
                   enable_asserts=True, num_devices=W)
    xT = nc.dram_tensor("xT", [D, NT], BF16, kind="ExternalInput").ap()
    wqkv = nc.dram_tensor("wqkv", [D, CQKV], BF16, kind="ExternalInput").ap()
    bqkv = nc.dram_tensor("bqkv", [2 * HL * 128], F32, kind="ExternalInput").ap()
    wout = nc.dram_tensor("wout", [D, D], BF16, kind="ExternalInput").ap()
    maskneg = nc.dram_tensor("maskneg", [128, 128], BF16, kind="ExternalInput").ap()
    bvbc = nc.dram_tensor("bvbc", [128, 2 * HL * Dh], F32, kind="ExternalInput").ap()
    boutbc = nc.dram_tensor("boutbc", [128, D], F32, kind="ExternalInput").ap()
    # rows [(b*2+half)*TOKH ...): tokens [half*HT + c*TOKH ...) of batch b
    out = nc.dram_tensor("out", [B * 2 * TOKH, D], F32, kind="ExternalOutput").ap()

    xT_v = xT.rearrange("(ko p) t -> p ko t", p=128)
    wqkv_v = wqkv.rearrange("(ko p) c -> p ko c", p=128)
    wout_v = wout.rearrange("(ko p) c -> p ko c", p=128)

    with tile.TileContext(nc) as tc:
        with tc.tile_pool(name="persist", bufs=1) as persist, \
             tc.tile_pool(name="dram", bufs=1, space="DRAM") as dram_pool:
            mask_sb = persist.tile([128, 128], BF16, name="mask")   # 0 / -1e9
            ones8_sb = persist.tile([128, 2, 128], BF16, name="ones8")
            bqk_sb = persist.tile([128, 2 * HL], F32, name="bqk")
            expb_sb = persist.tile([128, 1], F32, name="expb")
            bv_sb = persist.tile([128, 2 * HL * Dh], F32, name="bv")  # (hl tb d)
            bout_sb = persist.tile([128, D], F32, name="bout")
            wqkv_sb = [persist.tile([128, CQKV], BF16, name=f"wqkv{ko}")
                       for ko in range(KO)]
            wout_sb = [persist.tile([128, D], BF16, name=f"wout{ko}")
                       for ko in range(KO)]
            qT = [persist.tile([128, HL, T], BF16, name=f"qT{b}") for b in range(B)]
            kT = [persist.tile([128, HL, T], BF16, name=f"kT{b}") for b in range(B)]
            v = [persist.tile([128, HL, T // 128, Dh], BF16, name=f"v{b}")
                 for b in range(B)]

            # small constants + qkv weights first (needed immediately)
            nc.gpsimd.memset(expb_sb[:], EXPB)
            nc.gpsimd.memset(ones8_sb[:], 1.0)
            nc.sync.dma_start(mask_sb[:], maskneg)
            nc.sync.dma_start(bqk_sb[:], bqkv.rearrange("(cc p) -> p cc", p=128))
            nc.sync.dma_start(bv_sb[:], bvbc)

            a2a_in = [[dram_pool.tile([W, HL * 128, TOKH], BF16, name=f"a2a_in{b}{h}")
                       for h in range(2)] for b in range(B)]
            a2a_out = [[dram_pool.tile([W, HL * 128, TOKH], BF16, name=f"a2a_out{b}{h}")
                        for h in range(2)] for b in range(B)]

            with tc.tile_pool(name="x_pool", bufs=2) as x_pool, \
                 tc.tile_pool(name="ex_pool", bufs=3) as ex_pool, \
                 tc.tile_pool(name="rden_pool", bufs=2) as rden_pool, \
                 tc.tile_pool(name="attc_pool", bufs=3) as attc_pool, \
                 tc.tile_pool(name="attall_pool", bufs=4) as attall_pool, \
                 tc.tile_pool(name="o_pool", bufs=3) as o_pool, \
                 tc.tile_pool(name="proj_psum", bufs=2, space="PSUM") as proj_psum, \
                 tc.tile_pool(name="s_psum", bufs=2, space="PSUM") as s_psum, \
                 tc.tile_pool(name="av_psum", bufs=2, space="PSUM") as av_psum, \
                 tc.tile_pool(name="dout_psum", bufs=2, space="PSUM") as dout_psum:

                def prefetch_x(b, ci):
                    t0 = b * T + ci * TC
                    x_sb = x_pool.tile([128, KO, TC], BF16, name="x_sb")
                    nc.sync.dma_start(x_sb[:], xT_v[:, :, t0:t0 + TC])
                    return x_sb

                def emit_proj_chunk(qkv, b, ci, x_pre=None):
                    """Project one 512-token chunk of batch b into qT/kT/v."""
                    qTb, kTb, vb = qkv
                    x_sb = x_pre if x_pre is not None else prefetch_x(b, ci)
                    for ccp in range(2):            # 0: Q (hl0,hl1), 1: K
                        ps = proj_psum.tile([128, 2, TC], F32, name="proj_ps")
                        for i in range(2):
                            cc = ccp * 2 + i
                            for ko in range(KO):
                                nc.tensor.matmul(
                                    ps[:, i, :],
                                    wqkv_sb[ko][:, cc * 128:(cc + 1) * 128],
                                    x_sb[:, ko, :],
                                    start=(ko == 0), stop=(ko == KO - 1))
                        dest = qTb if ccp == 0 else kTb
                        for i in range(2):
                            nc.vector.tensor_scalar_add(
                                dest[:, i, ci * TC:(ci + 1) * TC], ps[:, i, :],
                                bqk_sb[:, ccp * 2 + i:ccp * 2 + i + 1])
                    ps = proj_psum.tile([128, 2, TC], F32, name="proj_ps")
                    for tb in range(TC // 128):
                        for ko in range(KO):
                            nc.tensor.matmul(
                                ps[:, tb, 0:256],
                                x_sb[:, ko, tb * 128:(tb + 1) * 128],
                                wqkv_sb[ko][:, 2 * HL * 128:],
                                start=(ko == 0), stop=(ko == KO - 1))
                    vidx = ci * (TC // 128)
                    nc.vector.tensor_tensor(
                        vb[:, :, vidx:vidx + 2, :],
                        ps[:, :, 0:256].rearrange("p tb (hl d) -> p hl tb d",
                                                  hl=HL),
                        bv_sb[:].rearrange("p (hl tb d) -> p hl tb d",
                                           hl=HL, tb=2),
                        ADD)

                def emit_attn_group(qkv, b, hl, qc):
                    qTb, kTb, vb = qkv
                    """One (head, q-chunk) group: S^T -> exp -> P^T V, denom via
                    ones-matmul; normalized att^T chunk DMAed to a2a_in.

                    Off-diagonal k-blocks are processed in pairs as fp8
                    DoubleRow matmuls (2x PE rate); diagonal blocks get an
                    additive -1e9 causal mask on the fp32 scores pre-exp."""
                    q0 = qc * QC
                    nkb = (qc + 1) * (QC // 128)
                    ndiag = QC // 128
                    npair = (nkb - ndiag) // 2
                    ps_av = av_psum.tile([128, QC], F32, name="ps_av")
                    ps_d = dout_psum.tile([128, QC], F32, name="ps_do")
                    units = [("pair", 2 * i) for i in range(npair)] \
                        + [("diag", 2 * npair + j) for j in range(ndiag)]
                    exs = {}

                    def emit_S_unit(u):
                        kind, kb = u
                        if kind == "pair":
                            ex2 = ex_pool.tile([128, 2, QC], BF16, name="ex2")
                            for t in range(2):
                                ps_s = s_psum.tile([128, QC], F32, name="ps_s")
                                nc.tensor.matmul(
                                    ps_s[:],
                                    kTb[:, hl, (kb + t) * 128:(kb + t + 1) * 128],
                                    qTb[:, hl, q0:q0 + QC],
                                    start=True, stop=True)
                                nc.scalar.activation(
                                    ex2[:, t, :], ps_s[:],
                                    mybir.ActivationFunctionType.Exp,
                                    scale=SCALE, bias=expb_sb[:])
                            exs[u] = ex2
                        else:
                            vs = (kb - qc * ndiag) * 128
                            ps_s = s_psum.tile([128, QC], F32, name="ps_s")
                            nc.tensor.matmul(
                                ps_s[:, vs:], kTb[:, hl, kb * 128:(kb + 1) * 128],
                                qTb[:, hl, q0 + vs:q0 + QC], start=True, stop=True)
                            nc.vector.tensor_tensor(
                                ps_s[:, vs:vs + 128], ps_s[:, vs:vs + 128],
                                mask_sb[:], ADD)
                            ex = ex_pool.tile([128, QC], BF16, name="ex")
                            nc.scalar.activation(
                                ex[:, vs:], ps_s[:, vs:],
                                mybir.ActivationFunctionType.Exp,
                                scale=SCALE, bias=expb_sb[:])
                            exs[u] = (ex, vs)

                    def emit_PV_unit(u, first, last):
                        kind, kb = u
                        if kind == "pair":
                            ex2 = exs[u]
                            nc.tensor.matmul(
                                ps_av[:], vb[:, hl, kb, :], ex2[:, 0, :],
                                start=first, stop=False)
                            nc.tensor.matmul(
                                ps_av[:], vb[:, hl, kb + 1, :], ex2[:, 1, :],
                                start=False, stop=last)
                            nc.tensor.matmul(
                                ps_d[:], ones8_sb[:, 0, :], ex2[:, 0, :],
                                start=first, stop=False)
                            nc.tensor.matmul(
                                ps_d[:], ones8_sb[:, 1, :], ex2[:, 1, :],
                                start=False, stop=last)
                        else:
                            ex, vs = exs[u]
                            nc.tensor.matmul(
                                ps_av[:, vs:], vb[:, hl, kb, :], ex[:, vs:],
                                start=first, stop=last)
                            nc.tensor.matmul(
                                ps_d[:, vs:], ones8_sb[:, 0, :], ex[:, vs:],
                                start=first, stop=last)

                    emit_S_unit(units[0])
                    for j in range(1, len(units)):
                        emit_S_unit(units[j])
                        emit_PV_unit(units[j - 1], j == 1, False)
                    emit_PV_unit(units[-1], len(units) == 1, True)

                    rden = rden_pool.tile([128, QC], F32, name="rden")
                    nc.vector.reciprocal_approx_fast(rden[:], ps_d[:])
                    attc = attc_pool.tile([128, QC], BF16, name="attc")
                    nc.vector.tensor_tensor(attc[:], ps_av[:], rden[:], MULT)
                    h = qc // 2
                    view = a2a_in[b][h].rearrange(
                        "(hh rr) (hl p) t -> p hl hh rr t",
                        hh=2, rr=W // 2, hl=HL, p=128)
                    nc.gpsimd.dma_start(
                        view[:, hl, qc % 2],
                        attc[:].rearrange("p (rr t) -> p rr t", rr=W // 2))

                def emit_a2a(b, h):
                    nc.gpsimd.collective_compute(
                        "AllToAll", mybir.AluOpType.bypass,
                        replica_groups=[list(range(W))],
                        ins=[a2a_in[b][h][:].opt()], outs=[a2a_out[b][h][:].opt()])

                def emit_attall(b, h, slot):
                    ga = attall_pool.tile([128, KO, TOKH], BF16, name="attall")
                    nc.sync.dma_start(
                        ga[:],
                        a2a_out[b][h].rearrange("r (hl p) t -> p (r hl) t",
                                                hl=HL, p=128))
                    slot[(b, h)] = ga

                def emit_outproj(b, h, slot):
                    ga = slot[(b, h)]
                    for colc in range(D // 512):
                        ps_o = dout_psum.tile([128, 512], F32, name="ps_do")
                        for ko in range(KO):
                            nc.tensor.matmul(
                                ps_o[:], ga[:, ko, :],
                                wout_sb[ko][:, colc * 512:(colc + 1) * 512],
                                start=(ko == 0), stop=(ko == KO - 1))
                        o_sb = o_pool.tile([128, 512], F32, name="o_sb")
                        nc.vector.tensor_tensor(
                            o_sb[:], ps_o[:],
                            bout_sb[:, colc * 512:(colc + 1) * 512], ADD)
                        nc.sync.dma_start(
                            out[(b * 2 + h) * TOKH:(b * 2 + h + 1) * TOKH,
                                colc * 512:(colc + 1) * 512],
                            o_sb[:])

                ga = {}
                # x chunk 0 DMA first (1MB, gates the first matmul), then the
                # wqkv tiles; wout (8MB) is held back until proj chunk 2's
                # output exists so it can't steal startup HBM bandwidth
                x0_sb = x_pool.tile([128, KO, TC], BF16, name="x_sb")
                nc.sync.dma_start(x0_sb[:, 0:KO // 2], xT_v[:, 0:KO // 2, 0:TC])
                nc.scalar.dma_start(x0_sb[:, KO // 2:], xT_v[:, KO // 2:, 0:TC])
                x_pre = {(0, 0): x0_sb}
                for ko in range(KO):
                    nc.gpsimd.dma_start(wqkv_sb[ko][:], wqkv_v[:, ko, :])

                def emit_wout_load(qT0, ci):
                    # Tile hoists dependency-free DMAs to t=0, which would let
                    # this 8.5MB steal startup HBM bandwidth from the critical
                    # x/wqkv transfers. Gate each pair of tiles behind a write
                    # that depends on batch-0's qT chunk ci (WAW on the DMA),
                    # spreading the load across the whole batch-0 phase.
                    tq = ci * TC + 1
                    if ci == 0:
                        nc.vector.tensor_scalar_add(
                            bout_sb[:, 0:1], qT0[:, 0, tq:tq + 1], 0.0)
                        nc.scalar.dma_start(bout_sb[:], boutbc)
                    for ko in (2 * ci, 2 * ci + 1):
                        nc.vector.tensor_scalar_add(
                            wout_sb[ko][:, 0:1], qT0[:, 0, tq:tq + 1], 0.0)
                        nc.scalar.dma_start(wout_sb[ko][:], wout_v[:, ko, :])

                qkv0 = (qT[0], kT[0], v[0])
                qkv1 = (qT[1], kT[1], v[1])
                # batch 0: proj + attention interleaved, A2As fire mid-phase
                for ci in range(NTC_B):
                    emit_proj_chunk(qkv0, 0, ci, x_pre.get((0, ci)))
                    emit_wout_load(qT[0], ci)
                    if ci % 2 == 1:
                        qc = ci // 2
                        emit_attn_group(qkv0, 0, 0, qc)
                        emit_attn_group(qkv0, 0, 1, qc)
                        if qc == 1:
                            emit_a2a(0, 0)
                        if qc == 3:
                            emit_a2a(0, 1)
                # batch 1: same; out-projections all run in the tail
                for ci in range(NTC_B):
                    emit_proj_chunk(qkv1, 1, ci)
                    if ci == 2:
                        emit_attall(0, 0, ga)
                    if ci == 4:
                        emit_attall(0, 1, ga)
                    if ci % 2 == 1:
                        qc = ci // 2
                        emit_attn_group(qkv1, 1, 0, qc)
                        emit_attn_group(qkv1, 1, 1, qc)
                        if qc == 1:
                            emit_a2a(1, 0)
                        if qc == 3:
                            emit_a2a(1, 1)
                # tail: batch-0's out-projections are independent of batch-1's
                # A2As, so ~35us of PE work hides the last A2A's latency
                # before outproj(1,*) needs its data
                emit_attall(1, 0, ga)
                emit_attall(1, 1, ga)
                emit_outproj(0, 0, ga)
                emit_outproj(0, 1, ga)
                emit_outproj(1, 0, ga)
                emit_outproj(1, 1, ga)
    nc.compile()
    return nc


_CACHED_NC = None


def kernel(x, Wqkv, bqkv, Wout, bout):
    global _CACHED_NC
    x = np.asarray(x, dtype=np.float32)
    Wqkv = np.asarray(Wqkv, dtype=np.float32)
    bqkv = np.asarray(bqkv, dtype=np.float32)
    Wout = np.asarray(Wout, dtype=np.float32)
    bout = np.asarray(bout, dtype=np.float32)

    if _CACHED_NC is None:
        _CACHED_NC = _build()
    nc = _CACHED_NC

    bf16 = ml_dtypes.bfloat16
    xT = np.ascontiguousarray(x.reshape(NT, D).T).astype(bf16)   # [D, NT]
    wq4 = Wqkv.reshape(D, 3, H, Dh)                 # col = (which, head, dh)
    bq4 = bqkv.reshape(3, H, Dh)
    kl = np.arange(128)[:, None]
    jl = np.arange(128)[None, :]
    masktri = (jl >= kl).astype(bf16)
    wout_bf = Wout.astype(bf16)
    boutbc = np.tile(bout[None, :], (128, 1)).astype(np.float32)

    in_maps = []
    for c in range(W):
        wshard = np.ascontiguousarray(
            wq4[:, :, HL * c:HL * c + HL, :].reshape(D, CQKV)).astype(bf16)
        bshard_qk = np.ascontiguousarray(
            bq4[0:2, HL * c:HL * c + HL, :].reshape(2 * HL * 128)
        ).astype(np.float32)
        bshard_v = bq4[2, HL * c:HL * c + HL, :]                  # [HL, Dh]
        bvbc = np.ascontiguousarray(np.broadcast_to(
            bshard_v.reshape(1, HL, 1, Dh), (128, HL, 2, Dh)
        ).reshape(128, 2 * HL * Dh)).astype(np.float32)
        in_maps.append({
            "xT": xT, "wqkv": wshard, "bqkv": bshard_qk,
            "wout": wout_bf, "masktri": masktri,
            "ones": np.ones((128, 128), bf16),
            "bvbc": bvbc,
            "boutbc": boutbc,
        })

    res = run_bass_kernel_spmd(nc, in_maps, core_ids=list(range(W)))
    # res[c]["out"] rows [(b*2+h)*TOKH ...) = tokens [h*HT + c*TOKH ...) of batch b
    full = np.empty((B, T, D), np.float32)
    for c in range(W):
        for b in range(B):
            for h in range(2):
                full[b, h * HT + c * TOKH:h * HT + (c + 1) * TOKH] = \
                    res.results[c]["out"][(b * 2 + h) * TOKH:(b * 2 + h + 1) * TOKH]
    return full
